# revision 11
# baseline (speedup 1.0000x reference)
"""Expert-parallel MoE (top-k routing + SwiGLU experts) for 8 Trainium2 cores.

Strategy
--------
- Host computes the (tiny) gate: logits = x @ gate_w (+ noise * noise_weight),
  top-k selection, sparse softmax weights.  0.03% of total FLOPs.
- Expert-parallel: core e owns expert e's weights.  Host gathers the tokens
  routed to expert e (padded to a common capacity C), core e runs a dense
  fused SwiGLU MLP over them:  out = (x@w1+b1) * silu(x@w2+b2) @ wp + bp,
  scaled by the per-token gate weight (folded into the final evacuation).
- Host scatter-adds the 8 partial outputs back to token positions.

Device kernel (tokens always on the free axis; bf16 matmul inputs with
f32 PSUM accumulation):
- x^T [D,C] bf16 resident in SBUF, loaded as per-(k,block) chunks so the
  first matmuls start after ~256KB instead of ~8MB (kills the head bubble).
- 16 dep-free warmup matmuls on a zeroed tile bring the PE HAM clock to
  8/8 while the first DMAs land.
- loop over 8 h-groups of 512 rows of H, streaming that group's w1/w2/wp
  as per-k 128KB slices spread over the scalar/gpsimd DMA queues;
  per token block of 512:
    hT[128h, tok] = (w1g.T @ xT + b1) * silu(w2g.T @ xT + b2)   (bf16)
    out_acc[128d, tok] += wpg.T @ hT          (PSUM acc over the 512 h)
  g=0 folds bp via the ACTIVATE bias; g=7 fuses the (acc + psB) * gate
  epilogue per (block, dm) and streams the output DMA immediately, so
  the kernel tail is just the last block's epilogue.
"""

import sys
import numpy as np

sys.path.insert(0, "/opt/trn_rl_repo")

D = 1024
H = 4096
E = 8
KD = D // 128          # 8 k-tiles over D
G = 8                  # h-groups
HJ = 4                 # 128-row h-tiles per group (G*HJ*128 == H)
TB = 512               # token block (matmul output must fit one PSUM bank)
WARMUP_MMS = 16

_NC_CACHE = {}


def _blocks(C):
    blocks = []
    o = 0
    while o < C:
        blocks.append((o, min(TB, C - o)))
        o += TB
    return blocks


def _build(C):
    import concourse.mybir as mybir
    import concourse.tile as tile
    from concourse import bacc

    f32 = mybir.dt.float32
    bf16 = mybir.dt.bfloat16
    ACT = mybir.ActivationFunctionType
    ALU = mybir.AluOpType

    nc = bacc.Bacc()
    xeT = nc.dram_tensor("xeT", [D, C], bf16, kind="ExternalInput")
    w1 = nc.dram_tensor("w1", [D, H], bf16, kind="ExternalInput")
    w2 = nc.dram_tensor("w2", [D, H], bf16, kind="ExternalInput")
    wp = nc.dram_tensor("wp", [H, D], bf16, kind="ExternalInput")
    b1 = nc.dram_tensor("b1", [H], f32, kind="ExternalInput")
    b2 = nc.dram_tensor("b2", [H], f32, kind="ExternalInput")
    bp = nc.dram_tensor("bp", [D], f32, kind="ExternalInput")
    gwb = nc.dram_tensor("gwb", [128, C], f32, kind="ExternalInput")
    outT = nc.dram_tensor("outT", [D, C], f32, kind="ExternalOutput")

    blocks = _blocks(C)
    NB = len(blocks)

    # strided views
    xTr = xeT.rearrange("(kt p) c -> kt p c", p=128)                  # [8,128,C]
    w1r = w1.rearrange("(k p) (g c) -> g k p c", p=128, c=512)        # [8,8,128,512]
    w2r = w2.rearrange("(k p) (g c) -> g k p c", p=128, c=512)
    wpr = wp.rearrange("(g hk p) c -> g hk p c", p=128, hk=HJ)        # [8,4,128,1024]
    b1r = b1.rearrange("(m p) -> p m", p=128)                         # [128,32]
    b2r = b2.rearrange("(m p) -> p m", p=128)
    bpr = bp.rearrange("(m p) -> p m", p=128)                         # [128,8]

    with tile.TileContext(nc) as tc:
        with (
            tc.tile_pool(name="pwu", bufs=1) as pwu,
            tc.tile_pool(name="pw12", bufs=2) as pw12,
            tc.tile_pool(name="pwp", bufs=2) as pwp,
            tc.tile_pool(name="px", bufs=1) as px,
            tc.tile_pool(name="pht", bufs=2) as pht,
            tc.tile_pool(name="ps2", bufs=3) as ps2,
            tc.tile_pool(name="pacc", bufs=1) as pacc,
            tc.tile_pool(name="pst", bufs=4) as pst,
            tc.tile_pool(name="pgw", bufs=1) as pgw,
            tc.tile_pool(name="pb", bufs=1) as pb,
            tc.tile_pool(name="pp", bufs=8, space="PSUM") as pp,
        ):
            # -- PE warmup: dep-free matmuls on a scratch tile (contents
            # irrelevant, result never read); they run while the first
            # input DMAs land so the real MM stream starts with the HAM
            # clock at 8/8.
            wut = pwu.tile([128, TB], bf16, tag="wu")
            nc.vector.memset(wut[:], 0)
            wups = pp.tile([128, TB], f32, tag="ps")
            for _ in range(WARMUP_MMS):
                nc.tensor.matmul(wups[:], wut[:, 0:128], wut[:],
                                 start=True, stop=True)

            # biases (tiny, SWDGE queue)
            b1s = pb.tile([128, G * HJ], f32, tag="b1s")
            nc.gpsimd.dma_start(b1s[:], b1r)
            b2s = pb.tile([128, G * HJ], f32, tag="b2s")
            nc.gpsimd.dma_start(b2s[:], b2r)
            bps = pb.tile([128, KD], f32, tag="bps")
            nc.gpsimd.dma_start(bps[:], bpr)

            # resident x^T in per-(k, block) chunks: the first (g0, b0)
            # matmul only waits for one 128KB chunk, not all of x.
            xb = [[None] * NB for _ in range(KD)]
            for bi, (bo, bs) in enumerate(blocks):
                for kt in range(KD):
                    t = px.tile([128, bs], bf16, tag=f"x{kt}_{bi}",
                                name=f"x{kt}_{bi}")
                    nc.sync.dma_start(t[:], xTr[kt, :, bo:bo + bs])
                    xb[kt][bi] = t

            # gate weights broadcast [128, C]; needed only at g == G-1
            # (DMA issued after g0's wp slices, below)
            gwt = pgw.tile([128, C], f32, tag="gw")

            oacc = [pacc.tile([128, C], f32, tag=f"o{dm}", name=f"oacc{dm}")
                    for dm in range(KD)]

            # ---- main: h-groups of 512, software-pipelined so block
            # b's psB chains (gated on its ht tiles) sit a full h-phase
            # behind their producers in the PE FIFO ----
            def h_phase(g, bi, bs, w1t, w2t):
                hts = []
                for hj in range(HJ):
                    hm = g * HJ + hj
                    co = hj * 128
                    # ps2t first: silu overlaps the ps1 chain and both
                    # PSUM banks release sooner
                    ps2t = pp.tile([128, bs], f32, tag="ps")
                    for k in range(KD):
                        nc.tensor.matmul(
                            ps2t[:], w2t[k][:, co:co + 128], xb[k][bi][:],
                            start=(k == 0), stop=(k == KD - 1))
                    s2 = ps2.tile([128, bs], f32, tag="s2")
                    nc.scalar.activation(s2[:], ps2t[:], ACT.Silu,
                                         bias=b2s[:, hm:hm + 1])
                    ps1 = pp.tile([128, bs], f32, tag="ps")
                    for k in range(KD):
                        nc.tensor.matmul(
                            ps1[:], w1t[k][:, co:co + 128], xb[k][bi][:],
                            start=(k == 0), stop=(k == KD - 1))
                    ht = pht.tile([128, bs], bf16, tag=f"h{hj}")
                    nc.vector.scalar_tensor_tensor(
                        ht[:], ps1[:], b1s[:, hm:hm + 1], s2[:],
                        op0=ALU.add, op1=ALU.mult)
                    hts.append(ht)
                return hts

            def dm_phase(g, bo, bs, wpt, hts):
                for dm in range(KD):
                    psB = pp.tile([128, bs], f32, tag="ps")
                    for hk in range(HJ):
                        nc.tensor.matmul(
                            psB[:], wpt[hk][:, dm * 128:dm * 128 + 128],
                            hts[hk][:], start=(hk == 0), stop=(hk == HJ - 1))
                    osl = oacc[dm][:, bo:bo + bs]
                    if g == 0:
                        # oacc = psB + bp; split between ACT and DVE so
                        # neither engine paces the DMA-fed first group
                        if dm % 2 == 0:
                            nc.scalar.activation(osl, psB[:], ACT.Identity,
                                                 bias=bps[:, dm:dm + 1])
                        else:
                            nc.vector.tensor_scalar_add(osl, psB[:],
                                                        bps[:, dm:dm + 1])
                    elif g < G - 1:
                        nc.vector.tensor_add(osl, osl, psB[:])
                    else:
                        # fused epilogue: out = (oacc + psB) * gate,
                        # streamed out per (block, dm)
                        st = pst.tile([128, bs], f32, tag="st")
                        nc.vector.tensor_add(st[:], osl, psB[:])
                        nc.vector.tensor_mul(st[:], st[:],
                                             gwt[:, bo:bo + bs])
                        nc.sync.dma_start(
                            outT[dm * 128:(dm + 1) * 128, bo:bo + bs],
                            st[:])

            for g in range(G):
                w1t, w2t = [], []
                for k in range(KD):
                    t1 = pw12.tile([128, 512], bf16, tag=f"w1k{k}")
                    nc.scalar.dma_start(t1[:], w1r[g, k])
                    w1t.append(t1)
                for k in range(KD):
                    t2 = pw12.tile([128, 512], bf16, tag=f"w2k{k}")
                    nc.scalar.dma_start(t2[:], w2r[g, k])
                    w2t.append(t2)
                wpt = []
                for hk in range(HJ):
                    t3 = pwp.tile([128, 1024], bf16, tag=f"wp{hk}")
                    nc.gpsimd.dma_start(t3[:], wpr[g, hk])
                    wpt.append(t3)
                if g == 1:
                    nc.gpsimd.dma_start(gwt[:], gwb[:])

                prev = None  # (bo, bs, hts) of the previous block
                for bi, (bo, bs) in enumerate(blocks):
                    hts = h_phase(g, bi, bs, w1t, w2t)
                    if prev is not None:
                        dm_phase(g, prev[0], prev[1], wpt, prev[2])
                    prev = (bo, bs, hts)
                dm_phase(g, prev[0], prev[1], wpt, prev[2])

    nc.finalize()
    return nc


def _route(x2d, noise2d, gate_w, noise_weight, kk):
    T = x2d.shape[0]
    logits = x2d @ gate_w
    logits = logits + noise2d * noise_weight[None, :]
    kk = int(kk)
    Ee = logits.shape[1]
    if kk >= Ee:
        sel = np.ones((T, Ee), dtype=bool)
    else:
        part = np.argpartition(-logits, kk - 1, axis=1)[:, :kk]
        sel = np.zeros((T, Ee), dtype=bool)
        sel[np.arange(T)[:, None], part] = True
    mx = logits.max(axis=1, keepdims=True)
    ex = np.exp(logits - mx, dtype=np.float32) * sel
    gw = ex / ex.sum(axis=1, keepdims=True)
    return sel, gw.astype(np.float32)


def _prep_maps(x2d, gw, idxs, C, w1, b1, w2, b2, wp, bp):
    import ml_dtypes
    bf16 = ml_dtypes.bfloat16
    in_maps = []
    for e in range(E):
        idx = idxs[e]
        n = len(idx)
        xeT = np.zeros((D, C), dtype=bf16)
        xeT[:, :n] = x2d[idx].T.astype(bf16)
        gwb = np.zeros((128, C), dtype=np.float32)
        gwb[:, :n] = gw[idx, e][None, :]
        in_maps.append({
            "xeT": xeT,
            "w1": np.ascontiguousarray(w1[e]).astype(bf16),
            "w2": np.ascontiguousarray(w2[e]).astype(bf16),
            "wp": np.ascontiguousarray(wp[e]).astype(bf16),
            "b1": np.ascontiguousarray(b1[e], dtype=np.float32),
            "b2": np.ascontiguousarray(b2[e], dtype=np.float32),
            "bp": np.ascontiguousarray(bp[e], dtype=np.float32),
            "gwb": gwb,
        })
    return in_maps


def kernel(**inputs):
    from concourse.bass_utils import run_bass_kernel_spmd

    x = np.asarray(inputs["x"], dtype=np.float32)
    noise = np.asarray(inputs["noise"], dtype=np.float32)
    gate_w = np.asarray(inputs["gate_w"], dtype=np.float32)
    noise_weight = np.asarray(inputs["noise_weight"], dtype=np.float32)
    w1 = np.asarray(inputs["w1"], dtype=np.float32)
    b1 = np.asarray(inputs["b1"], dtype=np.float32)
    w2 = np.asarray(inputs["w2"], dtype=np.float32)
    b2 = np.asarray(inputs["b2"], dtype=np.float32)
    wp = np.asarray(inputs["wp"], dtype=np.float32)
    bp = np.asarray(inputs["bp"], dtype=np.float32)
    kk = int(np.asarray(inputs["k"]))

    B, S, _ = x.shape
    T = B * S
    x2d = np.ascontiguousarray(x.reshape(T, D))
    noise2d = noise.reshape(T, E)

    sel, gw = _route(x2d, noise2d, gate_w, noise_weight, kk)
    idxs = [np.nonzero(sel[:, e])[0] for e in range(E)]
    maxn = max(len(i) for i in idxs)
    C = max(512, ((maxn + 63) // 64) * 64)

    if C not in _NC_CACHE:
        _NC_CACHE[C] = _build(C)
    nc = _NC_CACHE[C]

    in_maps = _prep_maps(x2d, gw, idxs, C, w1, b1, w2, b2, wp, bp)
    res = run_bass_kernel_spmd(nc, in_maps, core_ids=list(range(E))).results

    y2d = np.zeros((T, D), dtype=np.float32)
    for e in range(E):
        n = len(idxs[e])
        if n:
            y2d[idxs[e]] += res[e]["outT"][:, :n].T
    return y2d.reshape(B, S, D)
